# revision 8
# baseline (speedup 1.0000x reference)
"""Trainium2 Bass kernel for nn_KAN_NODE (Neural-ODE forecaster).

Strategy (8 cores, pure data parallel over batch B=2048 -> 256/core):
  * Only ~130 of 512 `past` rows are ever touched by the fixed-step RK4
    interpolation times (t = j/64) -- gather just those pairs by DMA and
    fold the linear interpolation + x-projection into one K=64 matmul per
    time point (weights = [W;W] stacked, pair pre-scaled by (1-w, w)).
  * ODE runs feature-major [128 feat, batch free], split into 2
    independent batch chains of FD=128 to pipeline the serial RK4 chain.
  * LayerNorm: mean-subtract fused into a PE matmul with the centering
    matrix C = I - 11^T/128; variance via Square(y_c + sqrt(eps)) (exact
    because y_c is mean-centered) + a J/256 matmul that also replicates
    the result across partitions; rsqrt via a custom 8-stage DVE op
    (two Newton steps seeded with the previous eval's rsqrt).
  * LN gain/bias, b1 folded into weights/silu-bias host side.  RK4
    combine accumulated in PSUM with pre-scaled W3 variants; state
    updates via the fused AFFINE_THEN_ADD DVE op.
  * Final MLP feature-major; eps output computed batch-major so the
    store DMA is contiguous; fb3 delivered by a K=1 ones matmul.
Only activation table set used is silu_and_others (no table switches).
"""

import sys
import numpy as np

for _p in ("/opt/trn_rl_repo",):
    if _p not in sys.path:
        sys.path.insert(0, _p)

# ---------------- problem constants (hardcoded per spec) ----------------
B, L, D_IN = 2048, 512, 32
PRED_LEN, COND, XPROJ, ODE_H, TDIM, HID = 96, 128, 128, 128, 128, 256
N_STEPS = 32
LN_EPS = 1e-5
NCORES = 8
BC = B // NCORES            # 256 batch rows per core
DT = 1.0 / N_STEPS
NJ = 2 * N_STEPS + 1        # 65 distinct interp time points t_j = j/64
OUTF = PRED_LEN * D_IN      # 3072
INF = OUTF + COND + TDIM    # 3328
NXCH = OUTF // 128          # 24 x_flat feature chunks
SQRT_EPS = float(np.sqrt(LN_EPS))

# interp: pos_j = t_j*(L-1); start row s_j (pairs are rows s_j, s_j+1)
_pos = (np.arange(NJ) / (NJ - 1)) * (L - 1)
_S = np.minimum(np.floor(_pos).astype(np.int64), L - 2)
_W = (_pos - _S).astype(np.float32)          # in [0,1]; j=64 -> s=510,w=1

_BUILT = None   # cache of (nc, RSQRT_OP)


def _register_rsqrt_op():
    import concourse.dve_ops as dve_ops
    from concourse.dve_spec import Spec, Src0, Src1, C0, sq, lower, _has_src1
    from concourse.dve_uop import DveOpSpec

    name = "RSQRT_NR2_ANT"
    for op in dve_ops.OPS:
        if op.name == name:
            return op

    y1 = Src0 * (C0 - Src1 * sq(Src0))
    body = y1 * (C0 - Src1 * sq(y1))

    def _ref(in0, in1, s0, s1, imm2):
        y = in0.astype(np.float32)
        v = in1.astype(np.float32)
        t = y * (s0 - v * y * y)
        return t * (s0 - v * t * t)

    spec = Spec(body=body, reference=_ref)
    row = dve_ops._CUSTOM_DVE_ROW_BASE + len(dve_ops.OPS)
    dve_ops._SUB_OPCODE_FOR_NAME[name] = row
    shas = {}
    for ver in ("v3", "v4"):
        try:
            uops = lower(spec, ver=ver)
            shas[ver] = DveOpSpec(
                name=name, opcode=row, uops=uops, rd1_en=_has_src1(spec)
            ).sha(ver)
        except Exception:
            pass
    op = dve_ops.DveOp(name, spec, subdim=False, uops_sha=shas)
    dve_ops.OPS.append(op)
    dve_ops.CUSTOM_DVE_SPECS[name] = spec
    return op


def _build():
    """Trace the Bass/Tile program once (same SPMD program for all cores)."""
    global _BUILT
    if _BUILT is not None:
        return _BUILT

    import concourse.bass as bass  # noqa: F401
    from concourse import bacc
    from concourse import mybir
    from concourse import tile

    RSQRT = _register_rsqrt_op()
    AF = mybir.ActivationFunctionType
    f32 = mybir.dt.float32
    Alu = mybir.AluOpType

    nc = bacc.Bacc(trn_type="TRN2")

    def din(name, shape):
        return nc.dram_tensor(name, list(shape), f32, kind="ExternalInput")

    # ---- DRAM inputs (per-core shards + replicated preprocessed weights)
    past_d = din("past", (BC, L, D_IN))
    xflat_d = din("xflat", (BC, OUTF))
    tembT_d = din("tembT", (TDIM, BC))
    rs0_d = din("rs0", (128, BC))
    wstackT_d = din("wstackT", (64, 128))
    cscale_d = din("cscale", (64, NJ))
    xpb_d = din("xpb", (128, 1))
    z0wT_d = din("z0wT", (128, 128))
    z0b_d = din("z0b", (128, 1))
    cmat_d = din("cmat", (128, 128))
    jhalf_d = din("jhalf", (128, 128))
    w1gT_d = din("w1gT", (128, 128))
    w1xT_d = din("w1xT", (128, 128))
    c1_d = din("c1", (128, 1))
    w2T_d = din("w2T", (128, 128))
    b2_d = din("b2", (128, 1))
    w3T_d = din("w3T", (128, 128))
    w3g1T_d = din("w3g1T", (128, 128))
    w3g2T_d = din("w3g2T", (128, 128))
    b3h_d = din("b3h", (128, 1))     # (dt/2)*b3
    sqeps_d = din("sqeps", (128, 1))
    b3d_d = din("b3d", (128, 1))     # dt*b3
    ident_d = din("ident", (128, 128))
    ones1_d = din("ones1", (1, 128))
    fw1T_d = din("fw1T", (INF, HID))
    fb1_d = din("fb1", (HID, 1))
    fw2T_d = din("fw2T", (HID, HID))
    fb2_d = din("fb2", (HID, 1))
    fw3T_d = din("fw3T", (HID, OUTF))
    fb3_d = din("fb3", (1, OUTF))
    eps_d = nc.dram_tensor("eps", [BC, OUTF], f32, kind="ExternalOutput")

    with tile.TileContext(nc) as tc:
        with (
            tc.tile_pool(name="const", bufs=1) as cp,
            tc.tile_pool(name="xt", bufs=1) as xtp,
            tc.tile_pool(name="state", bufs=2) as stp,
            tc.tile_pool(name="xfT", bufs=1) as xfp,
            tc.tile_pool(name="hbuf", bufs=1) as hbp,
        ):
            def cload(name, dram, shape):
                t = cp.tile(list(shape), f32, name=name, tag=name)
                nc.sync.dma_start(t[:], dram[tuple(slice(0, s) for s in shape)])
                return t

            wstackT = cload("wstackT_s", wstackT_d, (64, 128))
            cscale = cload("cscale_s", cscale_d, (64, NJ))
            xpb = cload("xpb_s", xpb_d, (128, 1))
            z0wT = cload("z0wT_s", z0wT_d, (128, 128))
            z0b = cload("z0b_s", z0b_d, (128, 1))
            cmat = cload("cmat_s", cmat_d, (128, 128))
            jhalf = cload("jhalf_s", jhalf_d, (128, 128))
            w1gT = cload("w1gT_s", w1gT_d, (128, 128))
            w1xT = cload("w1xT_s", w1xT_d, (128, 128))
            c1 = cload("c1_s", c1_d, (128, 1))
            w2T = cload("w2T_s", w2T_d, (128, 128))
            b2 = cload("b2_s", b2_d, (128, 1))
            w3T = cload("w3T_s", w3T_d, (128, 128))
            w3g1T = cload("w3g1T_s", w3g1T_d, (128, 128))
            w3g2T = cload("w3g2T_s", w3g2T_d, (128, 128))
            b3h = cload("b3h_s", b3h_d, (128, 1))
            sqeps = cload("sqeps_s", sqeps_d, (128, 1))
            b3d = cload("b3d_s", b3d_d, (128, 1))
            ident = cload("ident_s", ident_d, (128, 128))
            ones1 = cload("ones1_s", ones1_d, (1, 128))
            rs0 = cload("rs0_s", rs0_d, (128, BC))
            tembT = cload("tembT_s", tembT_d, (TDIM, BC))
            # fb1/fb2 halves (HID=256 > 128 partitions)
            fb1h = []
            fb2h = []
            for hh in range(2):
                t1 = cp.tile([128, 1], f32, name=f"fb1h{hh}", tag=f"fb1h{hh}")
                nc.sync.dma_start(t1[:], fb1_d[hh * 128:(hh + 1) * 128, :])
                fb1h.append(t1)
                t2 = cp.tile([128, 1], f32, name=f"fb2h{hh}", tag=f"fb2h{hh}")
                nc.sync.dma_start(t2[:], fb2_d[hh * 128:(hh + 1) * 128, :])
                fb2h.append(t2)

            fw1Tc = []
            for ci in range(INF // 128):   # 26 chunks
                t = cp.tile([128, HID], f32, name=f"fw1Tc{ci}", tag=f"fw1Tc{ci}")
                nc.sync.dma_start(t[:], fw1T_d[ci * 128:(ci + 1) * 128, :])
                fw1Tc.append(t)
            fw2Tr = []
            for ri in range(2):
                t = cp.tile([128, HID], f32, name=f"fw2Tr{ri}", tag=f"fw2Tr{ri}")
                nc.sync.dma_start(t[:], fw2T_d[ri * 128:(ri + 1) * 128, :])
                fw2Tr.append(t)
            fw3Tr = []
            for ri in range(2):
                t = cp.tile([128, OUTF], f32, name=f"fw3Tr{ri}", tag=f"fw3Tr{ri}")
                nc.sync.dma_start(t[:], fw3T_d[ri * 128:(ri + 1) * 128, :])
                fw3Tr.append(t)
            fb3 = cp.tile([1, OUTF], f32, name="fb3_s", tag="fb3_s")
            nc.sync.dma_start(fb3[:], fb3_d[:, :])

            xflatT = []
            for ci in range(NXCH):
                t = xfp.tile([128, BC], f32, name=f"xflatT{ci}", tag=f"xflatT{ci}")
                xflatT.append(t)

            with (
                tc.tile_pool(name="pairs", bufs=3) as prp,
                tc.tile_pool(name="xtps", bufs=2, space="PSUM") as xtps,
                tc.tile_pool(name="odeps", bufs=1, space="PSUM") as ops_,
            ):
                xt_tiles = {}

                def ensure_xt(j):
                    if j in xt_tiles:
                        return
                    s = int(_S[j])
                    pt = prp.tile([64, BC], f32, name=f"pair{j}", tag="pair")
                    src = past_d[:, s:s + 2, :].rearrange("b l d -> (l d) b")
                    nc.sync.dma_start(pt[:], src)
                    sc = prp.tile([64, BC], f32, name=f"psc{j}", tag="psc")
                    nc.vector.tensor_scalar(
                        sc[:], pt[:], cscale[:, j:j + 1], None, Alu.mult
                    )
                    xps = xtps.tile([128, BC], f32, name=f"xtps{j}", tag="xtps")
                    nc.tensor.matmul(xps[:], wstackT[:], sc[:],
                                     start=True, stop=True)
                    xt = xtp.tile([128, BC], f32, name=f"xt{j}", tag=f"xt{j}")
                    nc.scalar.activation(xt[:], xps[:], AF.Identity, bias=xpb[:, 0:1])
                    xt_tiles[j] = xt

                ensure_xt(0)

                # z0 = z0_w @ xt_0 + z0_b   (feature-major)
                z0ps = xtps.tile([128, BC], f32, name="z0ps", tag="xtps")
                nc.tensor.matmul(z0ps[:], z0wT[:], xt_tiles[0][:],
                                 start=True, stop=True)
                z_t = []
                y_t = [None, None]
                rs_prev = []
                zacc = [None, None]
                for c in range(2):
                    cs = slice(c * 128, (c + 1) * 128)
                    zt = stp.tile([128, 128], f32, name=f"zinit{c}", tag=f"z{c}")
                    nc.scalar.activation(zt[:], z0ps[:, cs], AF.Identity,
                                         bias=z0b[:, 0:1])
                    z_t.append(zt)
                    rs_prev.append(rs0[:, cs])

                A_E = [0.5 * DT, 0.5 * DT, DT]          # y-update scales
                BV_E = [b3h, b3h, b3d]                  # matching (a*b3) biases

                for k in range(N_STEPS):
                    for j in (2 * k, 2 * k + 1, 2 * k + 2):
                        ensure_xt(j)

                    # spread x_flat PE-transposes across the ODE
                    if k < NXCH:
                        for bh in range(2):
                            xb = prp.tile([128, 128], f32,
                                          name=f"xfb{k}_{bh}", tag="xfbm")
                            nc.sync.dma_start(
                                xb[:],
                                xflat_d[bh * 128:(bh + 1) * 128,
                                        k * 128:(k + 1) * 128])
                            tp = xtps.tile([128, 128], f32,
                                           name=f"tp{k}_{bh}", tag="xtps")
                            nc.tensor.transpose(tp[:], xb[:], ident[:])
                            nc.scalar.copy(
                                xflatT[k][:, bh * 128:(bh + 1) * 128], tp[:])

                    for e in range(4):
                        j = (2 * k, 2 * k + 1, 2 * k + 1, 2 * k + 2)[e]
                        for c in range(2):
                            cs = slice(c * 128, (c + 1) * 128)
                            y_in = z_t[c] if e == 0 else y_t[c]
                            # bank A per (chain,eval): yc|vh, ONE psum group
                            paT = ops_.tile([128, 256], f32,
                                            name=f"pa{c}_{k}_{e}",
                                            tag=f"pa{c}")
                            ycps = paT[:, 0:128]
                            vhps = paT[:, 128:256]
                            nc.tensor.matmul(ycps, cmat[:], y_in[:],
                                             start=True, stop=False,
                                             skip_group_check=True)
                            ycsq = stp.tile([128, 128], f32,
                                            name=f"sq{c}_{k}_{e}", tag=f"sq{c}")
                            nc.scalar.activation(ycsq[:], ycps, AF.Square,
                                                 bias=sqeps[:, 0:1])
                            nc.tensor.matmul(vhps, jhalf[:], ycsq[:],
                                             start=False, stop=True,
                                             skip_group_check=True)
                            rs = stp.tile([128, 128], f32,
                                          name=f"rs{c}_{k}_{e}", tag=f"rs{c}")
                            nc.vector._custom_dve(RSQRT, out=rs[:],
                                                  in0=rs_prev[c], in1=vhps,
                                                  s0=1.5)
                            rs_prev[c] = rs[:]
                            yn = stp.tile([128, 128], f32,
                                          name=f"yn{c}_{k}_{e}", tag=f"yn{c}")
                            nc.vector.tensor_tensor(yn[:], ycps, rs[:],
                                                    Alu.mult)
                            # bank B per (chain,eval): w1|w2|k, ONE group.
                            # w1g first: its yn dep chain-orders the bank
                            # re-zero after all prior-eval readers.
                            pbT = ops_.tile([128, 384], f32,
                                            name=f"pb{c}_{k}_{e}",
                                            tag=f"pb{c}")
                            w1ps = pbT[:, 0:128]
                            w2ps = pbT[:, 128:256]
                            kps = pbT[:, 256:384]
                            nc.tensor.matmul(w1ps, w1gT[:], yn[:],
                                             start=True, stop=False,
                                             skip_group_check=True)
                            nc.tensor.matmul(w1ps, w1xT[:],
                                             xt_tiles[j][:, cs],
                                             start=False, stop=False,
                                             skip_group_check=True)
                            a1 = stp.tile([128, 128], f32,
                                          name=f"a1{c}_{k}_{e}", tag=f"a1{c}")
                            nc.scalar.activation(a1[:], w1ps, AF.Silu,
                                                 bias=c1[:, 0:1])
                            nc.tensor.matmul(w2ps, w2T[:], a1[:],
                                             start=False, stop=(e == 3),
                                             skip_group_check=True)
                            a2 = stp.tile([128, 128], f32,
                                          name=f"a2{c}_{k}_{e}", tag=f"a2{c}")
                            nc.scalar.activation(a2[:], w2ps, AF.Silu,
                                                 bias=b2[:, 0:1])
                            if e == 0:
                                zacc[c] = ops_.tile([128, 128], f32,
                                                    name=f"za{c}_{k}",
                                                    tag=f"za{c}")
                            wz = w3g1T if e in (0, 3) else w3g2T
                            nc.tensor.matmul(zacc[c][:], wz[:], a2[:],
                                             start=(e == 0), stop=(e == 3),
                                             skip_group_check=True)
                            if e < 3:
                                nc.tensor.matmul(kps, w3T[:], a2[:],
                                                 start=False, stop=True,
                                                 skip_group_check=True)
                                ynew = stp.tile([128, 128], f32,
                                                name=f"y{c}_{k}_{e}",
                                                tag=f"y{c}")
                                nc.vector.affine_then_add(
                                    ynew[:], kps, z_t[c][:],
                                    scale=A_E[e], bias=BV_E[e][:, 0:1])
                                y_t[c] = ynew
                            else:
                                znew = stp.tile([128, 128], f32,
                                                name=f"z{c}_{k}", tag=f"z{c}")
                                nc.vector.affine_then_add(
                                    znew[:], zacc[c][:], z_t[c][:],
                                    scale=1.0, bias=b3d[:, 0:1])
                                z_t[c] = znew

            # ---------------- final MLP ----------------
            with tc.tile_pool(name="mlpps", bufs=1, space="PSUM") as mp:
                h1 = []
                for hh in range(2):
                    hs = slice(hh * 128, (hh + 1) * 128)
                    h1ps = mp.tile([128, BC], f32, name=f"h1ps{hh}",
                                   tag=f"h1ps{hh}")
                    nc.tensor.matmul(h1ps[:], fw1Tc[0][:, hs], xflatT[0][:],
                                     start=True, stop=False)
                    for ci in range(1, NXCH):
                        nc.tensor.matmul(h1ps[:], fw1Tc[ci][:, hs],
                                         xflatT[ci][:], start=False, stop=False)
                    for c in range(2):
                        cs = slice(c * 128, (c + 1) * 128)
                        nc.tensor.matmul(h1ps[:, cs], fw1Tc[NXCH][:, hs],
                                         z_t[c][:], start=False, stop=False)
                    nc.tensor.matmul(h1ps[:], fw1Tc[NXCH + 1][:, hs],
                                     tembT[:], start=False, stop=True)
                    h1t = hbp.tile([128, BC], f32, name=f"h1_{hh}",
                                   tag=f"h1_{hh}")
                    nc.scalar.activation(h1t[:], h1ps[:], AF.Silu,
                                         bias=fb1h[hh][:, 0:1])
                    h1.append(h1t)

                h2 = []
                for hh in range(2):
                    hs = slice(hh * 128, (hh + 1) * 128)
                    h2ps = mp.tile([128, BC], f32, name=f"h2ps{hh}",
                                   tag=f"h2ps{hh}")
                    nc.tensor.matmul(h2ps[:], fw2Tr[0][:, hs], h1[0][:],
                                     start=True, stop=False)
                    nc.tensor.matmul(h2ps[:], fw2Tr[1][:, hs], h1[1][:],
                                     start=False, stop=True)
                    h2t = hbp.tile([128, BC], f32, name=f"h2_{hh}",
                                   tag=f"h2_{hh}")
                    nc.scalar.activation(h2t[:], h2ps[:], AF.Silu,
                                         bias=fb2h[hh][:, 0:1])
                    h2.append(h2t)

                with tc.tile_pool(name="outps", bufs=2, space="PSUM") as op2, \
                        tc.tile_pool(name="outsb", bufs=3) as osb:
                    for bh in range(2):
                        bs = slice(bh * 128, (bh + 1) * 128)
                        for nt in range(OUTF // 512):
                            ns = slice(nt * 512, (nt + 1) * 512)
                            ops2 = op2.tile([128, 512], f32,
                                            name=f"ops{bh}_{nt}", tag="ops")
                            nc.tensor.matmul(ops2[:], ones1[:], fb3[:, ns],
                                             start=True, stop=False)
                            nc.tensor.matmul(ops2[:], h2[0][:, bs],
                                             fw3Tr[0][:, ns],
                                             start=False, stop=False)
                            nc.tensor.matmul(ops2[:], h2[1][:, bs],
                                             fw3Tr[1][:, ns],
                                             start=False, stop=True)
                            ot = osb.tile([128, 512], f32,
                                          name=f"ot{bh}_{nt}", tag="ot")
                            nc.scalar.copy(ot[:], ops2[:])
                            nc.sync.dma_start(eps_d[bs, ns], ot[:])

    nc.compile()
    _BUILT = nc
    return nc


def _host_inputs(x_t, past, t, xproj_w, xproj_b, z0_w, z0_b, ln_g, ln_b,
                 w1, b1, w2, b2, w3, b3, fw1, fb1, fw2, fb2, fw3, fb3):
    """Host-side staging: shard + precompute transposed/folded weights."""
    f = np.float32
    t = np.asarray(t)

    # time embedding (pure function of integer t) computed host side
    half = TDIM // 2
    freqs = np.exp(-f(np.log(10000.0)) *
                   np.arange(half, dtype=f) / f(half - 1)).astype(f)
    args = t.astype(f)[:, None] * freqs[None, :]
    temb = np.concatenate([np.sin(args), np.cos(args)], axis=1).astype(f)

    # rsqrt seed from host z0 (device refines with 2 Newton steps vs its
    # own exact variance, so seed precision is uncritical)
    xs0 = past[:, 0, :].astype(f) @ xproj_w.T.astype(f) + xproj_b
    z0h = xs0 @ z0_w.T.astype(f) + z0_b
    var0 = z0h.var(axis=1) + LN_EPS
    rs0 = (1.0 / np.sqrt(var0)).astype(f)               # (B,)

    cscale = np.zeros((64, NJ), f)
    cscale[:32, :] = (1.0 - _W)[None, :]
    cscale[32:, :] = _W[None, :]

    w1a = w1[:, :COND].astype(f)
    w1x = w1[:, COND:].astype(f)

    const = {
        "wstackT": np.ascontiguousarray(
            np.concatenate([xproj_w.T, xproj_w.T], axis=0)).astype(f),
        "cscale": cscale,
        "xpb": xproj_b.reshape(128, 1).astype(f),
        "z0wT": np.ascontiguousarray(z0_w.T).astype(f),
        "z0b": z0_b.reshape(128, 1).astype(f),
        "cmat": (np.eye(128, dtype=f) - f(1.0 / 128)),
        "jhalf": np.full((128, 128), 0.5 / 128, f),
        "w1gT": np.ascontiguousarray((w1a * ln_g[None, :]).T).astype(f),
        "w1xT": np.ascontiguousarray(w1x.T).astype(f),
        "c1": (w1a @ ln_b + b1).reshape(128, 1).astype(f),
        "w2T": np.ascontiguousarray(w2.T).astype(f),
        "b2": b2.reshape(128, 1).astype(f),
        "w3T": np.ascontiguousarray(w3.T).astype(f),
        "w3g1T": np.ascontiguousarray((w3 * f(DT / 6)).T).astype(f),
        "w3g2T": np.ascontiguousarray((w3 * f(DT / 3)).T).astype(f),
        "b3h": (b3 * f(DT / 2)).reshape(128, 1).astype(f),
        "sqeps": np.full((128, 1), SQRT_EPS, f),
        "b3d": (b3 * f(DT)).reshape(128, 1).astype(f),
        "ident": np.eye(128, dtype=f),
        "ones1": np.ones((1, 128), f),
        "fw1T": np.ascontiguousarray(fw1.T).astype(f),
        "fb1": fb1.reshape(HID, 1).astype(f),
        "fw2T": np.ascontiguousarray(fw2.T).astype(f),
        "fb2": fb2.reshape(HID, 1).astype(f),
        "fw3T": np.ascontiguousarray(fw3.T).astype(f),
        "fb3": fb3.reshape(1, OUTF).astype(f),
    }

    in_maps = []
    for c in range(NCORES):
        sl = slice(c * BC, (c + 1) * BC)
        m = dict(const)
        m["past"] = np.ascontiguousarray(past[sl]).astype(f)
        m["xflat"] = np.ascontiguousarray(
            x_t[sl].reshape(BC, OUTF)).astype(f)
        m["tembT"] = np.ascontiguousarray(temb[sl].T).astype(f)
        m["rs0"] = np.ascontiguousarray(
            np.broadcast_to(rs0[sl][None, :], (128, BC))).astype(f)
        in_maps.append(m)
    return in_maps


def kernel(**inputs) -> np.ndarray:
    from concourse.bass_utils import run_bass_kernel_spmd

    nc = _build()
    in_maps = _host_inputs(**{k: np.asarray(v) for k, v in inputs.items()})
    res = run_bass_kernel_spmd(nc, in_maps, core_ids=list(range(NCORES)))
    out = np.concatenate([r["eps"] for r in res.results], axis=0)
    return out.reshape(B, PRED_LEN, D_IN).astype(np.float32)
